# revision 32
# baseline (speedup 1.0000x reference)
"""AutoCorrelation (Autoformer) block on 8 TRN2 NeuronCores via Bass/Tile.

Math
----
The reference computes Q,K,V projections, per-head circular cross-correlation
R = irfft(rfft(Q) * conj(rfft(K))) over length L, then
mean_value[b,c] = R.mean(heads).mean(length), top-k over channels c, softmax
weights, a weighted sum of channel-index-shifted copies of V, and the output
projection.

Key identity: sum_l R[b,h,l,c] = (sum_t Q[b,h,t,c]) * (sum_s K[b,h,s,c]), so
no FFT is needed -- only column sums of q,k pushed through Wq,Wk.  The
shift-aggregation commutes with the fused projection Wvo = Wo @ Wv:
  out[b] = sum_d c[b,d] * roll(v[b], -d) @ Wvo.T + bvo
i.e. a 64-wide banded circular matmul applied to Y = v @ Wvo.T.

Plan (two device launches; all heavy compute on device):
  kernel A: per-core column sums of q,k shards (DMA-bound, 16MB/core).
  host:     exact (float64) 164-value top-k softmax -> 64-tap band matrices.
  kernel B: Y = v @ Wvo.T (fp16 in, fp32 accumulate) + banded shift + bias.

Sharding: core c handles batch b=c//2, length-half h=c%2 (2048 rows).
"""

from functools import lru_cache

import numpy as np

import concourse.bass as bass
import concourse.mybir as mybir
import concourse.tile as tile
from concourse import bacc
from concourse.bass_utils import run_bass_kernel_spmd

F32 = mybir.dt.float32
F16 = mybir.dt.float16

B, L, D = 4, 4096, 1024
H, DK = 16, D // 16
NCORES = 8
LH = L // 2                  # rows per core (2048)
NT = LH // 128               # 16 output tiles of 128 rows
VROWS = LH + 128             # v rows loaded per core (halo for shifts < 64)
NVT = VROWS // 128           # 17 v tiles
TOPK = min(max(1, int(5 * np.log(L + 1))), DK)   # 41


# --------------------------------------------------------------------------
# kernel A: column sums of the q/k shard -> qksum (2, D)
# --------------------------------------------------------------------------
@lru_cache(maxsize=1)
def _build_reduce():
    from contextlib import ExitStack
    nc = bacc.Bacc("TRN2", target_bir_lowering=False, debug=False,
                   num_devices=NCORES)
    q_d = nc.dram_tensor("q_in", [LH, D], F32, kind="ExternalInput")
    k_d = nc.dram_tensor("k_in", [LH, D], F32, kind="ExternalInput")
    o_d = nc.dram_tensor("qksum", [2, D], F32, kind="ExternalOutput")

    with ExitStack() as ctx:
        tc = ctx.enter_context(tile.TileContext(nc))
        stream = ctx.enter_context(tc.tile_pool(name="stream", bufs=6))
        accp = ctx.enter_context(tc.tile_pool(name="accp", bufs=1))
        rows = ctx.enter_context(tc.tile_pool(name="rows", bufs=1))
        pcs = ctx.enter_context(tc.tile_pool(name="pcs", bufs=2, space="PSUM"))

        ones128 = accp.tile([128, 1], F32)
        nc.vector.memset(ones128, 1.0)

        srcs = [d.ap().rearrange("(p n) d -> p n d", p=128) for d in (q_d, k_d)]
        accs = []
        for r in range(2):
            acc = accp.tile([128, D], F32, tag=f"acc{r}", name=f"acc{r}")
            accs.append(acc)
        # interleave q/k chunks so neither add-chain stalls the other's DMA
        for i in range(16):
            for r in range(2):
                ch = stream.tile([128, D], F32, tag="chunk", name=f"c{i}_{r}")
                nc.sync.dma_start(out=ch, in_=srcs[r][:, i, :])
                if i == 0:
                    nc.vector.tensor_copy(accs[r], ch)
                else:
                    nc.vector.tensor_add(accs[r], accs[r], ch)
        for r in range(2):
            row = rows.tile([1, D], F32, tag=f"row{r}", name=f"row{r}")
            for nh in range(2):
                ps = pcs.tile([128, 512], F32, tag="cs", name=f"cs_{r}_{nh}")
                nc.tensor.matmul(ps[0:1, :], ones128,
                                 accs[r][:, nh * 512:(nh + 1) * 512],
                                 start=True, stop=True)
                nc.vector.tensor_copy(row[:, nh * 512:(nh + 1) * 512],
                                      ps[0:1, :])
            nc.sync.dma_start(out=o_d.ap()[r:r + 1, :], in_=row)
    nc.compile()
    return nc


# --------------------------------------------------------------------------
# kernel B: Y = v @ Wvo.T (fp16 matmuls, fp32 psum), banded shift-combine,
# bias, store.  v arrives pre-transposed (j-major) in fp16, so the PE runs
# one dense back-to-back matmul stream with no on-device layout work.
# --------------------------------------------------------------------------
NCH = (NVT + 3) // 4          # 5 chunks of m-blocks (4 blocks of 128 each)


@lru_cache(maxsize=1)
def _build_main():
    from contextlib import ExitStack
    nc = bacc.Bacc("TRN2", target_bir_lowering=False, debug=False,
                   num_devices=NCORES)
    vt_d = nc.dram_tensor("vt16", [D, VROWS], F16, kind="ExternalInput")
    w_d = nc.dram_tensor("wvot16", [D, D], F16, kind="ExternalInput")
    bvo_d = nc.dram_tensor("bvo", [1, D], F32, kind="ExternalInput")
    b1_d = nc.dram_tensor("b1h", [128, 128], F16, kind="ExternalInput")
    b2_d = nc.dram_tensor("b2h", [64, 128], F16, kind="ExternalInput")
    o_d = nc.dram_tensor("out", [LH, D], F32, kind="ExternalOutput")

    with ExitStack() as ctx:
        tc = ctx.enter_context(tile.TileContext(nc))

        singles = ctx.enter_context(tc.tile_pool(name="singles", bufs=1))
        ysb = ctx.enter_context(tc.tile_pool(name="ysb", bufs=NVT))
        outp = ctx.enter_context(tc.tile_pool(name="outp", bufs=3))
        py = ctx.enter_context(tc.tile_pool(name="py", bufs=4, space="PSUM"))
        po = ctx.enter_context(tc.tile_pool(name="po", bufs=4, space="PSUM"))

        def bcast(dram_t, parts, cols):
            ap = dram_t.ap()
            return bass.AP(tensor=ap.tensor, offset=ap.offset,
                           ap=[[0, parts], [1, cols]])

        b1 = singles.tile([128, 128], F16)
        nc.sync.dma_start(out=b1, in_=b1_d.ap())
        b2 = singles.tile([64, 128], F16)
        nc.sync.dma_start(out=b2, in_=b2_d.ap())
        bvo_rep = singles.tile([128, D], F32)
        nc.sync.dma_start(out=bvo_rep, in_=bcast(bvo_d, 128, D))

        # interleave weight + first-chunk v loads so PE can start early
        wsrc = w_d.ap().rearrange("(c p) n -> c p n", p=128)
        vsrc = vt_d.ap().rearrange("(c p) m -> c p m", p=128)
        wvot = []
        vchunk = [[None] * NCH for _ in range(8)]

        def load_vchunk(jc, ch):
            m0 = ch * 512
            mw = min(512, VROWS - m0)
            t = singles.tile([128, mw], F16, tag=f"v{jc}_{ch}",
                             name=f"v{jc}_{ch}")
            nc.sync.dma_start(out=t, in_=vsrc[jc][:, m0:m0 + mw])
            vchunk[jc][ch] = t

        for jc in range(8):
            w = singles.tile([128, D], F16, tag=f"w{jc}", name=f"w{jc}")
            nc.sync.dma_start(out=w, in_=wsrc[jc])
            wvot.append(w)
            load_vchunk(jc, 0)
        for ch in range(1, NCH):
            for jc in range(8):
                load_vchunk(jc, ch)

        odst = o_d.ap()
        y_tiles = []

        def emit_band(ib):
            osb = outp.tile([128, D], F32, tag="osb", name=f"osb{ib}")
            for nh in range(2):
                ps = po.tile([128, 512], F32, tag="op", name=f"po{ib}_{nh}")
                nc.tensor.matmul(ps, b1,
                                 y_tiles[ib][:, nh * 512:(nh + 1) * 512],
                                 start=True, stop=False)
                nc.tensor.matmul(ps, b2,
                                 y_tiles[ib + 1][0:64, nh * 512:(nh + 1) * 512],
                                 start=False, stop=True)
                nc.vector.tensor_add(osb[:, nh * 512:(nh + 1) * 512], ps,
                                     bvo_rep[:, nh * 512:(nh + 1) * 512])
            nc.sync.dma_start(out=odst[ib * 128:(ib + 1) * 128, :], in_=osb)

        # band block ib is emitted right after y[ib+1] so output DMAs spread
        # across the kernel instead of bursting at the end
        for it in range(NVT):
            rows = 128 if it < NVT - 1 else 64
            ch, off = it // 4, (it % 4) * 128
            y = ysb.tile([128, D], F16, tag="y", name=f"y{it}")
            for nh in range(2):
                ps = py.tile([128, 512], F32, tag="yp", name=f"py{it}_{nh}")
                for jc in range(8):
                    nc.tensor.matmul(ps[0:rows],
                                     vchunk[jc][ch][:, off:off + rows],
                                     wvot[jc][:, nh * 512:(nh + 1) * 512],
                                     start=(jc == 0), stop=(jc == 7))
                nc.scalar.copy(y[0:rows, nh * 512:(nh + 1) * 512], ps[0:rows])
            y_tiles.append(y)
            if it >= 1:
                emit_band(it - 1)
        emit_band(NT - 1)
    nc.compile()
    return nc


# --------------------------------------------------------------------------
# host glue
# --------------------------------------------------------------------------
def _taps_from_sums(qs, ks, Wq, bq, Wk, bk):
    """Exact (float64) tap weights c (B, DK) from q/k column sums (B, D)."""
    Qs = qs @ Wq.T + L * bq                       # (B, D)
    Ks = ks @ Wk.T + L * bk
    mv = (Qs.reshape(B, H, DK) * Ks.reshape(B, H, DK)).mean(axis=1) / L
    mbar = mv.mean(axis=0)
    idx = np.argsort(-mbar)[:TOPK]
    sel = mv[:, idx]
    w = np.exp(sel - sel.max(axis=1, keepdims=True))
    w /= w.sum(axis=1, keepdims=True)
    c = np.zeros((B, DK))
    for j, d in enumerate(idx):
        c[:, d] = w[:, j]
    return c


def _band_mats(c_b):
    """B1[k,m] = c[k-m] (0<=k-m<64), B2[k',m] = c[128+k'-m]."""
    b1 = np.zeros((128, 128), np.float32)
    b2 = np.zeros((64, 128), np.float32)
    cf = c_b.astype(np.float32)
    for d in range(DK):
        w = cf[d]
        if w == 0.0:
            continue
        idx = np.arange(0, 128 - d)
        b1[idx + d, idx] = w
        idx2 = np.arange(128 - d, 128)
        b2[idx2 + d - 128, idx2] = w
    return b1, b2


def _devices_ok():
    try:
        import jax
        return len(jax.devices()) >= NCORES
    except Exception:
        return False


def _subproc_main(in_path, out_path):
    data = dict(np.load(in_path))
    out = kernel(**data)
    np.save(out_path, out)


def _kernel_via_subprocess(**inputs):
    """The caller's jax is pinned to a platform without the 8 NeuronCores
    (e.g. cpu for the reference); run the device launches in a fresh
    interpreter where the axon platform can initialize."""
    import os
    import subprocess
    import sys
    import tempfile
    d = tempfile.mkdtemp(prefix="ac_kernel_")
    in_path = os.path.join(d, "in.npz")
    out_path = os.path.join(d, "out.npy")
    np.savez(in_path, **{k: np.asarray(v) for k, v in inputs.items()})
    env = dict(os.environ)
    env.pop("JAX_PLATFORMS", None)
    kdir = os.path.dirname(os.path.abspath(__file__))
    code = (f"import sys; sys.path.insert(0, {kdir!r}); "
            f"import kernel; kernel._subproc_main({in_path!r}, {out_path!r})")
    subprocess.run([sys.executable, "-c", code], env=env, check=True)
    return np.load(out_path)


def kernel(q, k, v, Wq, bq, Wk, bk, Wv, bv, Wo, bo):
    if not _devices_ok():
        return _kernel_via_subprocess(q=q, k=k, v=v, Wq=Wq, bq=bq, Wk=Wk,
                                      bk=bk, Wv=Wv, bv=bv, Wo=Wo, bo=bo)
    f = np.float32
    q = np.asarray(q, f)
    k = np.asarray(k, f)
    v = np.asarray(v, f)

    # ---- launch A: q/k column sums -----------------------------------
    nc_a = _build_reduce()
    maps_a = []
    for c in range(NCORES):
        b, h = c // 2, c % 2
        l0 = h * LH
        maps_a.append({
            "q_in": np.ascontiguousarray(q[b, l0:l0 + LH]),
            "k_in": np.ascontiguousarray(k[b, l0:l0 + LH]),
        })
    res_a = run_bass_kernel_spmd(nc_a, maps_a, list(range(NCORES)))

    qs = np.zeros((B, D), np.float64)
    ks = np.zeros((B, D), np.float64)
    for c in range(NCORES):
        b = c // 2
        qs[b] += res_a.results[c]["qksum"][0].astype(np.float64)
        ks[b] += res_a.results[c]["qksum"][1].astype(np.float64)

    # ---- host: exact taps + band matrices ----------------------------
    taps = _taps_from_sums(
        qs, ks,
        np.asarray(Wq, np.float64), np.asarray(bq, np.float64),
        np.asarray(Wk, np.float64), np.asarray(bk, np.float64))

    wvo = np.asarray(Wo, np.float64) @ np.asarray(Wv, np.float64)
    wvot16 = np.ascontiguousarray(wvo.T.astype(np.float16))
    bvo = (np.asarray(bo, np.float64)
           + np.asarray(Wo, np.float64) @ np.asarray(bv, np.float64))
    bvo = bvo.astype(f).reshape(1, D)

    # ---- launch B: main compute --------------------------------------
    nc_b = _build_main()
    bands = [_band_mats(taps[b]) for b in range(B)]
    maps_b = []
    vt16 = {}
    for b in range(B):
        vt16[b] = np.ascontiguousarray(v[b].T.astype(np.float16))
    for c in range(NCORES):
        b, h = c // 2, c % 2
        l0 = h * LH
        vrows = np.arange(l0, l0 + VROWS) % L
        maps_b.append({
            "vt16": np.ascontiguousarray(vt16[b][:, vrows]),
            "wvot16": wvot16, "bvo": bvo,
            "b1h": bands[b][0].astype(np.float16),
            "b2h": bands[b][1].astype(np.float16),
        })
    res_b = run_bass_kernel_spmd(nc_b, maps_b, list(range(NCORES)))

    out = np.empty((B, L, D), np.float32)
    for c in range(NCORES):
        b, h = c // 2, c % 2
        out[b, h * LH:(h + 1) * LH] = res_b.results[c]["out"]
    return out


# revision 33
# speedup vs baseline: 1.0494x; 1.0494x over previous
"""AutoCorrelation (Autoformer) block on 8 TRN2 NeuronCores via Bass/Tile.

Math
----
The reference computes Q,K,V projections, per-head circular cross-correlation
R = irfft(rfft(Q) * conj(rfft(K))) over length L, then
mean_value[b,c] = R.mean(heads).mean(length), top-k over channels c, softmax
weights, a weighted sum of channel-index-shifted copies of V, and the output
projection.

Key identity: sum_l R[b,h,l,c] = (sum_t Q[b,h,t,c]) * (sum_s K[b,h,s,c]), so
no FFT is needed -- only column sums of q,k pushed through Wq,Wk.  The
shift-aggregation commutes with the fused projection Wvo = Wo @ Wv:
  out[b] = sum_d c[b,d] * roll(v[b], -d) @ Wvo.T + bvo
i.e. a 64-wide banded circular matmul applied to Y = v @ Wvo.T.

Plan (two device launches; all heavy compute on device):
  kernel A: per-core column sums of q,k shards (DMA-bound, 16MB/core).
  host:     exact (float64) 164-value top-k softmax -> 64-tap band matrices.
  kernel B: Y = v @ Wvo.T (fp16 in, fp32 accumulate) + banded shift + bias.

Sharding: core c handles batch b=c//2, length-half h=c%2 (2048 rows).
"""

from functools import lru_cache

import numpy as np

import concourse.bass as bass
import concourse.mybir as mybir
import concourse.tile as tile
from concourse import bacc
from concourse.bass_utils import run_bass_kernel_spmd

F32 = mybir.dt.float32
F16 = mybir.dt.float16

B, L, D = 4, 4096, 1024
H, DK = 16, D // 16
NCORES = 8
LH = L // 2                  # rows per core (2048)
NT = LH // 128               # 16 output tiles of 128 rows
VROWS = LH + 128             # v rows loaded per core (halo for shifts < 64)
NVT = VROWS // 128           # 17 v tiles
TOPK = min(max(1, int(5 * np.log(L + 1))), DK)   # 41


# --------------------------------------------------------------------------
# kernel A: column sums of the q/k shard -> qksum (2, D)
# --------------------------------------------------------------------------
@lru_cache(maxsize=1)
def _build_reduce():
    from contextlib import ExitStack
    nc = bacc.Bacc("TRN2", target_bir_lowering=False, debug=False,
                   num_devices=NCORES)
    q_d = nc.dram_tensor("q_in", [LH, D], F32, kind="ExternalInput")
    k_d = nc.dram_tensor("k_in", [LH, D], F32, kind="ExternalInput")
    o_d = nc.dram_tensor("qksum", [2, D], F32, kind="ExternalOutput")

    with ExitStack() as ctx:
        tc = ctx.enter_context(tile.TileContext(nc))
        stream = ctx.enter_context(tc.tile_pool(name="stream", bufs=6))
        accp = ctx.enter_context(tc.tile_pool(name="accp", bufs=1))
        rows = ctx.enter_context(tc.tile_pool(name="rows", bufs=1))
        pcs = ctx.enter_context(tc.tile_pool(name="pcs", bufs=2, space="PSUM"))

        ones128 = accp.tile([128, 1], F32)
        nc.vector.memset(ones128, 1.0)

        srcs = [d.ap().rearrange("(p n) d -> p n d", p=128) for d in (q_d, k_d)]
        accs = []
        for r in range(2):
            acc = accp.tile([128, D], F32, tag=f"acc{r}", name=f"acc{r}")
            accs.append(acc)
        # interleave q/k chunks so neither add-chain stalls the other's DMA
        for i in range(16):
            for r in range(2):
                ch = stream.tile([128, D], F32, tag="chunk", name=f"c{i}_{r}")
                nc.sync.dma_start(out=ch, in_=srcs[r][:, i, :])
                if i == 0:
                    nc.vector.tensor_copy(accs[r], ch)
                else:
                    nc.vector.tensor_add(accs[r], accs[r], ch)
        for r in range(2):
            row = rows.tile([1, D], F32, tag=f"row{r}", name=f"row{r}")
            for nh in range(2):
                ps = pcs.tile([128, 512], F32, tag="cs", name=f"cs_{r}_{nh}")
                nc.tensor.matmul(ps[0:1, :], ones128,
                                 accs[r][:, nh * 512:(nh + 1) * 512],
                                 start=True, stop=True)
                nc.vector.tensor_copy(row[:, nh * 512:(nh + 1) * 512],
                                      ps[0:1, :])
            nc.sync.dma_start(out=o_d.ap()[r:r + 1, :], in_=row)
    nc.compile()
    return nc


# --------------------------------------------------------------------------
# kernel B: Y = v @ Wvo.T (fp16 matmuls, fp32 psum), banded shift-combine,
# bias, store.  v arrives pre-transposed (j-major) in fp16, so the PE runs
# one dense back-to-back matmul stream with no on-device layout work.
# --------------------------------------------------------------------------
NCH = (NVT + 3) // 4          # 5 chunks of m-blocks (4 blocks of 128 each)


@lru_cache(maxsize=1)
def _build_main():
    from contextlib import ExitStack
    nc = bacc.Bacc("TRN2", target_bir_lowering=False, debug=False,
                   num_devices=NCORES)
    vt_d = nc.dram_tensor("vt16", [D, VROWS], F16, kind="ExternalInput")
    w_d = nc.dram_tensor("wvot16", [D, D], F16, kind="ExternalInput")
    bvo_d = nc.dram_tensor("bvo", [1, D], F32, kind="ExternalInput")
    b1_d = nc.dram_tensor("b1h", [128, 128], F16, kind="ExternalInput")
    b2_d = nc.dram_tensor("b2h", [64, 128], F16, kind="ExternalInput")
    o_d = nc.dram_tensor("out", [LH, D], F32, kind="ExternalOutput")

    with ExitStack() as ctx:
        tc = ctx.enter_context(tile.TileContext(nc))

        singles = ctx.enter_context(tc.tile_pool(name="singles", bufs=1))
        ysb = ctx.enter_context(tc.tile_pool(name="ysb", bufs=NVT))
        outp = ctx.enter_context(tc.tile_pool(name="outp", bufs=3))
        py = ctx.enter_context(tc.tile_pool(name="py", bufs=4, space="PSUM"))
        po = ctx.enter_context(tc.tile_pool(name="po", bufs=4, space="PSUM"))

        def bcast(dram_t, parts, cols):
            ap = dram_t.ap()
            return bass.AP(tensor=ap.tensor, offset=ap.offset,
                           ap=[[0, parts], [1, cols]])

        b1 = singles.tile([128, 128], F16)
        nc.sync.dma_start(out=b1, in_=b1_d.ap())
        b2 = singles.tile([64, 128], F16)
        nc.sync.dma_start(out=b2, in_=b2_d.ap())
        bvo_rep = singles.tile([128, D], F32)
        nc.sync.dma_start(out=bvo_rep, in_=bcast(bvo_d, 128, D))

        # interleave weight + first-chunk v loads so PE can start early
        wsrc = w_d.ap().rearrange("(c p) n -> c p n", p=128)
        vsrc = vt_d.ap().rearrange("(c p) m -> c p m", p=128)
        wvot = []
        vchunk = [[None] * NCH for _ in range(8)]

        def load_vchunk(jc, ch):
            m0 = ch * 512
            mw = min(512, VROWS - m0)
            t = singles.tile([128, mw], F16, tag=f"v{jc}_{ch}",
                             name=f"v{jc}_{ch}")
            nc.sync.dma_start(out=t, in_=vsrc[jc][:, m0:m0 + mw])
            vchunk[jc][ch] = t

        for jc in range(8):
            w = singles.tile([128, D], F16, tag=f"w{jc}", name=f"w{jc}")
            nc.sync.dma_start(out=w, in_=wsrc[jc])
            wvot.append(w)
            load_vchunk(jc, 0)
        for ch in range(1, NCH):
            for jc in range(8):
                load_vchunk(jc, ch)

        y_tiles = []
        for it in range(NVT):
            rows = 128 if it < NVT - 1 else 64
            ch, off = it // 4, (it % 4) * 128
            y = ysb.tile([128, D], F16, tag="y", name=f"y{it}")
            for nh in range(2):
                ps = py.tile([128, 512], F32, tag="yp", name=f"py{it}_{nh}")
                for jc in range(8):
                    nc.tensor.matmul(ps[0:rows],
                                     vchunk[jc][ch][:, off:off + rows],
                                     wvot[jc][:, nh * 512:(nh + 1) * 512],
                                     start=(jc == 0), stop=(jc == 7))
                nc.scalar.copy(y[0:rows, nh * 512:(nh + 1) * 512], ps[0:rows])
            y_tiles.append(y)

        odst = o_d.ap()
        for ib in range(NT):
            osb = outp.tile([128, D], F32, tag="osb", name=f"osb{ib}")
            for nh in range(2):
                ps = po.tile([128, 512], F32, tag="op", name=f"po{ib}_{nh}")
                nc.tensor.matmul(ps, b1,
                                 y_tiles[ib][:, nh * 512:(nh + 1) * 512],
                                 start=True, stop=False)
                nc.tensor.matmul(ps, b2,
                                 y_tiles[ib + 1][0:64, nh * 512:(nh + 1) * 512],
                                 start=False, stop=True)
                nc.vector.tensor_add(osb[:, nh * 512:(nh + 1) * 512], ps,
                                     bvo_rep[:, nh * 512:(nh + 1) * 512])
            nc.sync.dma_start(out=odst[ib * 128:(ib + 1) * 128, :], in_=osb)
    nc.compile()
    return nc


# --------------------------------------------------------------------------
# host glue
# --------------------------------------------------------------------------
def _taps_from_sums(qs, ks, Wq, bq, Wk, bk):
    """Exact (float64) tap weights c (B, DK) from q/k column sums (B, D)."""
    Qs = qs @ Wq.T + L * bq                       # (B, D)
    Ks = ks @ Wk.T + L * bk
    mv = (Qs.reshape(B, H, DK) * Ks.reshape(B, H, DK)).mean(axis=1) / L
    mbar = mv.mean(axis=0)
    idx = np.argsort(-mbar)[:TOPK]
    sel = mv[:, idx]
    w = np.exp(sel - sel.max(axis=1, keepdims=True))
    w /= w.sum(axis=1, keepdims=True)
    c = np.zeros((B, DK))
    for j, d in enumerate(idx):
        c[:, d] = w[:, j]
    return c


def _band_mats(c_b):
    """B1[k,m] = c[k-m] (0<=k-m<64), B2[k',m] = c[128+k'-m]."""
    b1 = np.zeros((128, 128), np.float32)
    b2 = np.zeros((64, 128), np.float32)
    cf = c_b.astype(np.float32)
    for d in range(DK):
        w = cf[d]
        if w == 0.0:
            continue
        idx = np.arange(0, 128 - d)
        b1[idx + d, idx] = w
        idx2 = np.arange(128 - d, 128)
        b2[idx2 + d - 128, idx2] = w
    return b1, b2


def _devices_ok():
    try:
        import jax
        return len(jax.devices()) >= NCORES
    except Exception:
        return False


def _subproc_main(in_path, out_path):
    data = dict(np.load(in_path))
    out = kernel(**data)
    np.save(out_path, out)


def _kernel_via_subprocess(**inputs):
    """The caller's jax is pinned to a platform without the 8 NeuronCores
    (e.g. cpu for the reference); run the device launches in a fresh
    interpreter where the axon platform can initialize."""
    import os
    import subprocess
    import sys
    import tempfile
    d = tempfile.mkdtemp(prefix="ac_kernel_")
    in_path = os.path.join(d, "in.npz")
    out_path = os.path.join(d, "out.npy")
    np.savez(in_path, **{k: np.asarray(v) for k, v in inputs.items()})
    env = dict(os.environ)
    env.pop("JAX_PLATFORMS", None)
    kdir = os.path.dirname(os.path.abspath(__file__))
    code = (f"import sys; sys.path.insert(0, {kdir!r}); "
            f"import kernel; kernel._subproc_main({in_path!r}, {out_path!r})")
    subprocess.run([sys.executable, "-c", code], env=env, check=True)
    return np.load(out_path)


def kernel(q, k, v, Wq, bq, Wk, bk, Wv, bv, Wo, bo):
    if not _devices_ok():
        return _kernel_via_subprocess(q=q, k=k, v=v, Wq=Wq, bq=bq, Wk=Wk,
                                      bk=bk, Wv=Wv, bv=bv, Wo=Wo, bo=bo)
    f = np.float32
    q = np.asarray(q, f)
    k = np.asarray(k, f)
    v = np.asarray(v, f)

    # ---- launch A: q/k column sums -----------------------------------
    nc_a = _build_reduce()
    maps_a = []
    for c in range(NCORES):
        b, h = c // 2, c % 2
        l0 = h * LH
        maps_a.append({
            "q_in": np.ascontiguousarray(q[b, l0:l0 + LH]),
            "k_in": np.ascontiguousarray(k[b, l0:l0 + LH]),
        })
    res_a = run_bass_kernel_spmd(nc_a, maps_a, list(range(NCORES)))

    qs = np.zeros((B, D), np.float64)
    ks = np.zeros((B, D), np.float64)
    for c in range(NCORES):
        b = c // 2
        qs[b] += res_a.results[c]["qksum"][0].astype(np.float64)
        ks[b] += res_a.results[c]["qksum"][1].astype(np.float64)

    # ---- host: exact taps + band matrices ----------------------------
    taps = _taps_from_sums(
        qs, ks,
        np.asarray(Wq, np.float64), np.asarray(bq, np.float64),
        np.asarray(Wk, np.float64), np.asarray(bk, np.float64))

    wvo = np.asarray(Wo, np.float64) @ np.asarray(Wv, np.float64)
    wvot16 = np.ascontiguousarray(wvo.T.astype(np.float16))
    bvo = (np.asarray(bo, np.float64)
           + np.asarray(Wo, np.float64) @ np.asarray(bv, np.float64))
    bvo = bvo.astype(f).reshape(1, D)

    # ---- launch B: main compute --------------------------------------
    nc_b = _build_main()
    bands = [_band_mats(taps[b]) for b in range(B)]
    maps_b = []
    vt16 = {}
    for b in range(B):
        vt16[b] = np.ascontiguousarray(v[b].T.astype(np.float16))
    for c in range(NCORES):
        b, h = c // 2, c % 2
        l0 = h * LH
        vrows = np.arange(l0, l0 + VROWS) % L
        maps_b.append({
            "vt16": np.ascontiguousarray(vt16[b][:, vrows]),
            "wvot16": wvot16, "bvo": bvo,
            "b1h": bands[b][0].astype(np.float16),
            "b2h": bands[b][1].astype(np.float16),
        })
    res_b = run_bass_kernel_spmd(nc_b, maps_b, list(range(NCORES)))

    out = np.empty((B, L, D), np.float32)
    for c in range(NCORES):
        b, h = c // 2, c % 2
        out[b, h * LH:(h + 1) * LH] = res_b.results[c]["out"]
    return out
